# revision 17
# baseline (speedup 1.0000x reference)
"""MoE FFN (E=8 experts, top-2 routing, SwiGLU) on 8 TRN2 NeuronCores.

Strategy (expert-parallel, per sharding hint):
  - Host computes the tiny gate (x @ Wg, 0.07% of total FLOPs), top-2 routing
    and softmax combine weights. This IS the sharding step: tokens are
    dispatched (gathered) per expert, one expert per core.
  - Each core runs the SwiGLU FFN for its expert over its routed tokens in
    bf16 (fp32 accumulation in PSUM), scales rows by the combine weight.
  - Host scatter-adds the 8 per-expert outputs back into the full [T, D]
    output (the unshard step).

Device kernel layout (per core, capacity C tokens, padded with zeros):
  phase 1:  hT[hid, tok] = silu(W1.T x.T) * (W3.T x.T)
            lhsT = W1/W3 tile [128d, 128h] (stationary), rhs = xT [128d, ntok]
  phase 2:  y[tok, d] = (hT.T @ W2) * w_tok
            lhsT = hT tile [128h, 128tok] (stationary), rhs = W2 [128h, 512d]
            combine weight applied as per-partition scale during PSUM evict.
"""

import os
import sys

import numpy as np

for _p in ("/opt/trn_rl_repo",):
    if os.path.isdir(_p) and _p not in sys.path:
        sys.path.insert(0, _p)

import ml_dtypes

DIM = 1024
HID = 2048
E = 8
TOPK = 2
P = 128
NCORES = 8
TBS = 512  # moving-dim token chunk for phase 1

BF16 = ml_dtypes.bfloat16

_KERNEL_CACHE = {}
LAST_RESULT = None  # BassKernelResults of the most recent run (for test.py)


def _build(C, C_comp):
    import concourse.mybir as mybir
    import concourse.tile as tile
    from concourse import bacc

    f32 = mybir.dt.float32
    bf16 = mybir.dt.bfloat16
    AF = mybir.ActivationFunctionType

    KD = DIM // P   # 8  k-chunks over DIM
    KH = HID // P   # 16 k-chunks over HID
    NT = C // P     # token blocks of 128

    # phase-1 moving-dim chunks: cover only the C_comp real tokens. Chunk 0
    # is a full 512 so the PE has a long runway while weight DMAs land; the
    # remainder splits evenly into chunks >~210 rows so LDWEIGHTS stays
    # hidden behind each matmul.
    chunks = []
    remaining = C_comp
    while remaining > 768:
        chunks.append(TBS)
        remaining -= TBS
    if remaining > 512:
        h = remaining // 2
        chunks += [remaining - h, h]
    elif remaining:
        chunks.append(remaining)

    nc = bacc.Bacc(None, target_bir_lowering=False, debug=False)

    xT = nc.declare_dram_parameter("xT", [DIM, C], bf16, isOutput=False)
    w1 = nc.declare_dram_parameter("w1", [DIM, HID], bf16, isOutput=False)
    w3 = nc.declare_dram_parameter("w3", [DIM, HID], bf16, isOutput=False)
    w2 = nc.declare_dram_parameter("w2", [HID, DIM], bf16, isOutput=False)
    wv = nc.declare_dram_parameter("wv", [P, NT], f32, isOutput=False)
    out = nc.declare_dram_parameter("out", [C, DIM], f32, isOutput=True)

    with tile.TileContext(nc) as tc:
        with (
            tc.tile_pool(name="persist", bufs=1) as const,
            tc.tile_pool(name="psA", bufs=3, space="PSUM") as psA,
            tc.tile_pool(name="psY", bufs=1, space="PSUM") as psY,
            tc.tile_pool(name="sil", bufs=3) as sil_pool,
            tc.tile_pool(name="ysb", bufs=2) as y_pool,
        ):
            xT_sb = [const.tile([P, C], bf16, name=f"xT{k}", tag=f"xT{k}") for k in range(KD)]
            w1_sb = [const.tile([P, HID], bf16, name=f"w1{k}", tag=f"w1{k}") for k in range(KD)]
            w3_sb = [const.tile([P, HID], bf16, name=f"w3{k}", tag=f"w3{k}") for k in range(KD)]
            w2_sb = [const.tile([P, DIM], bf16, name=f"w2{k}", tag=f"w2{k}") for k in range(KH)]
            wv_sb = const.tile([P, NT], f32, tag="wv")
            hT_sb = [const.tile([P, C], bf16, name=f"hT{k}", tag=f"hT{k}") for k in range(KH)]
            zb = const.tile([P, 1], f32, tag="zb")
            nc.any.memset(zb[:], 0.0)
            if C_comp < C:
                # hT/out columns past the last real token are never computed;
                # zero them so every out row is well-defined (host drops them)
                for k in range(KH):
                    nc.any.memset(hT_sb[k][:, C_comp:], 0.0)
            warm = const.tile([P, 256], bf16, tag="warm")
            nc.any.memset(warm[:], 0.0)

            # PE warmup: ~3us of dummy matmuls so the HAM clock-gate opens
            # (and the PE isn't idle) while the first input DMAs land.
            for _ in range(10):
                wp = psA.tile([P, TBS], f32, tag="ph1", bufs=4, name="warmp")
                nc.tensor.matmul(wp[:, :256], lhsT=warm[:, :P], rhs=warm[:])

            # input DMAs, ordered to match the PE's consumption: chunk 0 does
            # an all-hid W1 sweep first (needs W1 + x cols[0:c0]), then the W3
            # sweep, so W3/w2 can stream in behind the first 30us of matmuls.
            # the ph1 sweep consumes hid blocks 0..15 in order, so W1's low
            # column half (hb 0..7) is all the PE needs to reach full speed;
            # the high half streams in behind it.
            HH = HID // 2
            c0 = min(chunks[0], C)
            nc.sync.dma_start(out=xT_sb[0][:, :c0], in_=xT[0:P, :c0])
            nc.sync.dma_start(out=w1_sb[0][:, :P], in_=w1[0:P, :P])
            nc.sync.dma_start(out=w1_sb[0][:, P:HH], in_=w1[0:P, P:HH])
            for k in range(1, KD):
                nc.sync.dma_start(out=w1_sb[k][:, :HH], in_=w1[k * P:(k + 1) * P, :HH])
                nc.sync.dma_start(out=xT_sb[k][:, :c0], in_=xT[k * P:(k + 1) * P, :c0])
            for k in range(KD):
                nc.sync.dma_start(out=w1_sb[k][:, HH:], in_=w1[k * P:(k + 1) * P, HH:])
            for k in range(KD):
                nc.sync.dma_start(out=w3_sb[k][:, :HH], in_=w3[k * P:(k + 1) * P, :HH])
            for k in range(KD):
                nc.sync.dma_start(out=w3_sb[k][:, HH:], in_=w3[k * P:(k + 1) * P, HH:])
            if c0 < C:
                for k in range(KD):
                    nc.sync.dma_start(
                        out=xT_sb[k][:, c0:], in_=xT[k * P:(k + 1) * P, c0:]
                    )
            for k in range(KH):
                nc.sync.dma_start(out=w2_sb[k][:], in_=w2[k * P:(k + 1) * P, :])
            nc.sync.dma_start(out=wv_sb[:], in_=wv[:, :])

            def mm_sweep(dst_psum, w_sb, hb, t0, n):
                for k in range(KD):
                    nc.tensor.matmul(
                        dst_psum[:, :n],
                        lhsT=w_sb[k][:, hb * P:(hb + 1) * P],
                        rhs=xT_sb[k][:, t0:t0 + n],
                        start=(k == 0),
                        stop=(k == KD - 1),
                    )

            # phase 1: hT[hid, tok] over the C_comp real tokens
            # chunk 0: separate W1 sweep (stage silu(h1) as bf16) then W3 sweep
            n0 = chunks[0]
            slu = [
                const.tile([P, n0], bf16, name=f"slu{hb}", tag=f"slu{hb}")
                for hb in range(KH)
            ]
            for hb in range(KH):
                ph1 = psA.tile([P, TBS], f32, tag="ph1", bufs=4)
                mm_sweep(ph1, w1_sb, hb, 0, n0)
                sil = sil_pool.tile([P, TBS], f32, tag="sil")
                nc.scalar.activation(sil[:, :n0], ph1[:, :n0], AF.Sigmoid, bias=zb[:])
                nc.vector.tensor_mul(slu[hb][:], sil[:, :n0], ph1[:, :n0])
            for hb in range(KH):
                ph3 = psA.tile([P, TBS], f32, tag="ph3", bufs=2)
                mm_sweep(ph3, w3_sb, hb, 0, n0)
                nc.vector.tensor_mul(hT_sb[hb][:, :n0], slu[hb][:], ph3[:, :n0])

            # remaining chunks: fused per-hid-block ph1/ph3
            t0 = n0
            for n in chunks[1:]:
                for hb in range(KH):
                    ph1 = psA.tile([P, TBS], f32, tag="ph1", bufs=4)
                    ph3 = psA.tile([P, TBS], f32, tag="ph3", bufs=2)
                    mm_sweep(ph1, w1_sb, hb, t0, n)
                    mm_sweep(ph3, w3_sb, hb, t0, n)
                    # silu(h1)*h3 = sigmoid(h1)*h1*h3 (Silu isn't in CoreSim)
                    sil = sil_pool.tile([P, TBS], f32, tag="sil")
                    sg2 = sil_pool.tile([P, TBS], f32, tag="sg2")
                    nc.scalar.activation(sil[:, :n], ph1[:, :n], AF.Sigmoid, bias=zb[:])
                    nc.vector.tensor_mul(sg2[:, :n], sil[:, :n], ph1[:, :n])
                    nc.vector.tensor_mul(
                        hT_sb[hb][:, t0:t0 + n], sg2[:, :n], ph3[:, :n]
                    )
                t0 += n

            # phase 2: y[tok, d] scaled by combine weight
            for t in range(NT):
                py0 = psY.tile([P, 512], f32, tag="py0")
                py1 = psY.tile([P, 512], f32, tag="py1")
                for k in range(KH):
                    nc.tensor.matmul(
                        py0[:],
                        lhsT=hT_sb[k][:, t * P:(t + 1) * P],
                        rhs=w2_sb[k][:, 0:512],
                        start=(k == 0),
                        stop=(k == KH - 1),
                    )
                for k in range(KH):
                    nc.tensor.matmul(
                        py1[:],
                        lhsT=hT_sb[k][:, t * P:(t + 1) * P],
                        rhs=w2_sb[k][:, 512:1024],
                        start=(k == 0),
                        stop=(k == KH - 1),
                    )
                y0 = y_pool.tile([P, 512], f32, tag="y0")
                y1 = y_pool.tile([P, 512], f32, tag="y1")
                nc.scalar.activation(y0[:], py0[:], AF.Copy, scale=wv_sb[:, t:t + 1])
                nc.scalar.activation(y1[:], py1[:], AF.Copy, scale=wv_sb[:, t:t + 1])
                nc.sync.dma_start(out=out[t * P:(t + 1) * P, 0:512], in_=y0[:])
                nc.sync.dma_start(out=out[t * P:(t + 1) * P, 512:1024], in_=y1[:])

    nc.compile()
    return nc


def _get_kernel(C, C_comp):
    key = (C, C_comp)
    nc = _KERNEL_CACHE.get(key)
    if nc is None:
        nc = _build(C, C_comp)
        _KERNEL_CACHE[key] = nc
    return nc


def _route(xt, Wg):
    """Host gate: returns per-expert (token_indices, combine_weights)."""
    scores = xt.astype(np.float32) @ Wg.astype(np.float32)          # [T, E]
    top2 = np.argpartition(-scores, 1, axis=1)[:, :2]               # [T, 2]
    vals = np.take_along_axis(scores, top2, axis=1)                 # [T, 2]
    vals = vals - vals.max(axis=1, keepdims=True)
    ev = np.exp(vals)
    sm = ev / ev.sum(axis=1, keepdims=True)                         # [T, 2]
    T = xt.shape[0]
    combine = np.zeros((T, E), dtype=np.float32)
    combine[np.arange(T)[:, None], top2] = sm
    idx = []
    wts = []
    for e in range(E):
        ie = np.nonzero(combine[:, e])[0]
        idx.append(ie)
        wts.append(combine[ie, e])
    return idx, wts


def kernel(x, Wg, W1, W3, W2):
    global LAST_RESULT
    from concourse import bass_utils

    orig_shape = x.shape
    orig_dtype = x.dtype
    xt = np.ascontiguousarray(np.asarray(x, dtype=np.float32).reshape(-1, DIM))
    T = xt.shape[0]

    idx, wts = _route(xt, np.asarray(Wg, dtype=np.float32))
    max_n = max(len(i) for i in idx)
    C = max(P, -(-max_n // P) * P)
    C_comp = max(1, max_n)
    NT = C // P

    nc = _get_kernel(C, C_comp)

    W1 = np.asarray(W1)
    W3 = np.asarray(W3)
    W2 = np.asarray(W2)
    in_maps = []
    for e in range(E):
        n_e = len(idx[e])
        xT_e = np.zeros((DIM, C), dtype=BF16)
        xT_e[:, :n_e] = np.ascontiguousarray(xt[idx[e]].T).astype(BF16)
        wv_pad = np.zeros(C, dtype=np.float32)
        wv_pad[:n_e] = wts[e]
        wv_e = np.ascontiguousarray(wv_pad.reshape(NT, P).T)  # [P, NT]
        in_maps.append(
            {
                "xT": xT_e,
                "w1": W1[e].astype(BF16),
                "w3": W3[e].astype(BF16),
                "w2": W2[e].astype(BF16),
                "wv": wv_e,
            }
        )

    res = bass_utils.run_bass_kernel_spmd(nc, in_maps, core_ids=list(range(NCORES)))
    LAST_RESULT = res

    out = np.zeros((T, DIM), dtype=np.float32)
    for e in range(E):
        n_e = len(idx[e])
        if n_e:
            out[idx[e]] += np.asarray(res.results[e]["out"][:n_e], dtype=np.float32)
    return out.reshape(orig_shape).astype(orig_dtype, copy=False)


# revision 18
# speedup vs baseline: 1.0331x; 1.0331x over previous
"""MoE FFN (E=8 experts, top-2 routing, SwiGLU) on 8 TRN2 NeuronCores.

Strategy (expert-parallel, per sharding hint):
  - Host computes the tiny gate (x @ Wg, 0.07% of total FLOPs), top-2 routing
    and softmax combine weights. This IS the sharding step: tokens are
    dispatched (gathered) per expert, one expert per core.
  - Each core runs the SwiGLU FFN for its expert over its routed tokens in
    bf16 (fp32 accumulation in PSUM), scales rows by the combine weight.
  - Host scatter-adds the 8 per-expert outputs back into the full [T, D]
    output (the unshard step).

Device kernel layout (per core, capacity C tokens, padded with zeros):
  phase 1:  hT[hid, tok] = silu(W1.T x.T) * (W3.T x.T)
            lhsT = W1/W3 tile [128d, 128h] (stationary), rhs = xT [128d, ntok]
  phase 2:  y[tok, d] = (hT.T @ W2) * w_tok
            lhsT = hT tile [128h, 128tok] (stationary), rhs = W2 [128h, 512d]
            combine weight applied as per-partition scale during PSUM evict.
"""

import os
import sys

import numpy as np

for _p in ("/opt/trn_rl_repo",):
    if os.path.isdir(_p) and _p not in sys.path:
        sys.path.insert(0, _p)

import ml_dtypes

DIM = 1024
HID = 2048
E = 8
TOPK = 2
P = 128
NCORES = 8
TBS = 512  # moving-dim token chunk for phase 1

BF16 = ml_dtypes.bfloat16

_KERNEL_CACHE = {}
LAST_RESULT = None  # BassKernelResults of the most recent run (for test.py)


def _build(C, C_comp):
    import concourse.mybir as mybir
    import concourse.tile as tile
    from concourse import bacc

    f32 = mybir.dt.float32
    bf16 = mybir.dt.bfloat16
    AF = mybir.ActivationFunctionType

    KD = DIM // P   # 8  k-chunks over DIM
    KH = HID // P   # 16 k-chunks over HID
    NT = C // P     # token blocks of 128

    # phase-1 moving-dim chunks: cover only the C_comp real tokens. Chunk 0
    # is a full 512 so the PE has a long runway while weight DMAs land; the
    # remainder splits evenly into chunks >~210 rows so LDWEIGHTS stays
    # hidden behind each matmul.
    chunks = []
    remaining = C_comp
    while remaining > 768:
        chunks.append(TBS)
        remaining -= TBS
    if remaining > 512:
        h = remaining // 2
        chunks += [remaining - h, h]
    elif remaining:
        chunks.append(remaining)

    nc = bacc.Bacc(None, target_bir_lowering=False, debug=False)

    xT = nc.declare_dram_parameter("xT", [DIM, C], bf16, isOutput=False)
    w1 = nc.declare_dram_parameter("w1", [DIM, HID], bf16, isOutput=False)
    w3 = nc.declare_dram_parameter("w3", [DIM, HID], bf16, isOutput=False)
    w2 = nc.declare_dram_parameter("w2", [HID, DIM], bf16, isOutput=False)
    wb = nc.declare_dram_parameter("wb", [P, C], f32, isOutput=False)
    out = nc.declare_dram_parameter("out", [DIM, C], f32, isOutput=True)

    with tile.TileContext(nc) as tc:
        with (
            tc.tile_pool(name="persist", bufs=1) as const,
            tc.tile_pool(name="psA", bufs=3, space="PSUM") as psA,
            tc.tile_pool(name="psY", bufs=1, space="PSUM") as psY,
            tc.tile_pool(name="sil", bufs=3) as sil_pool,
            tc.tile_pool(name="ysb", bufs=2) as y_pool,
        ):
            xT_sb = [const.tile([P, C], bf16, name=f"xT{k}", tag=f"xT{k}") for k in range(KD)]
            w1_sb = [const.tile([P, HID], bf16, name=f"w1{k}", tag=f"w1{k}") for k in range(KD)]
            w3_sb = [const.tile([P, HID], bf16, name=f"w3{k}", tag=f"w3{k}") for k in range(KD)]
            w2_sb = [const.tile([P, DIM], bf16, name=f"w2{k}", tag=f"w2{k}") for k in range(KH)]
            wb_sb = const.tile([P, C], f32, tag="wb")
            hT_sb = [const.tile([P, C], bf16, name=f"hT{k}", tag=f"hT{k}") for k in range(KH)]
            zb = const.tile([P, 1], f32, tag="zb")
            nc.any.memset(zb[:], 0.0)
            warm = const.tile([P, 256], bf16, tag="warm")
            nc.any.memset(warm[:], 0.0)

            # PE warmup: ~3us of dummy matmuls so the HAM clock-gate opens
            # (and the PE isn't idle) while the first input DMAs land.
            for _ in range(10):
                wp = psA.tile([P, TBS], f32, tag="ph1", bufs=4, name="warmp")
                nc.tensor.matmul(wp[:, :256], lhsT=warm[:, :P], rhs=warm[:])

            # input DMAs, ordered to match the PE's consumption: chunk 0 does
            # an all-hid W1 sweep first (needs W1 + x cols[0:c0]), then the W3
            # sweep, so W3/w2 can stream in behind the first 30us of matmuls.
            # the ph1 sweep consumes hid blocks 0..15 in order, so W1's low
            # column half (hb 0..7) is all the PE needs to reach full speed;
            # the high half streams in behind it.
            HH = HID // 2
            c0 = min(chunks[0], C)
            nc.sync.dma_start(out=xT_sb[0][:, :c0], in_=xT[0:P, :c0])
            nc.sync.dma_start(out=w1_sb[0][:, :P], in_=w1[0:P, :P])
            nc.sync.dma_start(out=w1_sb[0][:, P:HH], in_=w1[0:P, P:HH])
            for k in range(1, KD):
                nc.sync.dma_start(out=w1_sb[k][:, :HH], in_=w1[k * P:(k + 1) * P, :HH])
                nc.sync.dma_start(out=xT_sb[k][:, :c0], in_=xT[k * P:(k + 1) * P, :c0])
            for k in range(KD):
                nc.sync.dma_start(out=w1_sb[k][:, HH:], in_=w1[k * P:(k + 1) * P, HH:])
            for k in range(KD):
                nc.sync.dma_start(out=w3_sb[k][:, :HH], in_=w3[k * P:(k + 1) * P, :HH])
            for k in range(KD):
                nc.sync.dma_start(out=w3_sb[k][:, HH:], in_=w3[k * P:(k + 1) * P, HH:])
            if c0 < C:
                for k in range(KD):
                    nc.sync.dma_start(
                        out=xT_sb[k][:, c0:], in_=xT[k * P:(k + 1) * P, c0:]
                    )
            for k in range(KH):
                nc.sync.dma_start(out=w2_sb[k][:], in_=w2[k * P:(k + 1) * P, :])
            nc.sync.dma_start(out=wb_sb[:], in_=wb[:, :])

            def mm_sweep(dst_psum, w_sb, hb, t0, n):
                for k in range(KD):
                    nc.tensor.matmul(
                        dst_psum[:, :n],
                        lhsT=w_sb[k][:, hb * P:(hb + 1) * P],
                        rhs=xT_sb[k][:, t0:t0 + n],
                        start=(k == 0),
                        stop=(k == KD - 1),
                    )

            # phase 1: hT[hid, tok] over the C_comp real tokens
            # chunk 0: separate W1 sweep (stage silu(h1) as bf16) then W3 sweep
            n0 = chunks[0]
            slu = [
                const.tile([P, n0], bf16, name=f"slu{hb}", tag=f"slu{hb}")
                for hb in range(KH)
            ]
            for hb in range(KH):
                ph1 = psA.tile([P, TBS], f32, tag="ph1", bufs=4)
                mm_sweep(ph1, w1_sb, hb, 0, n0)
                sil = sil_pool.tile([P, TBS], f32, tag="sil")
                nc.scalar.activation(sil[:, :n0], ph1[:, :n0], AF.Sigmoid, bias=zb[:])
                nc.vector.tensor_mul(slu[hb][:], sil[:, :n0], ph1[:, :n0])
            for hb in range(KH):
                ph3 = psA.tile([P, TBS], f32, tag="ph3", bufs=2)
                mm_sweep(ph3, w3_sb, hb, 0, n0)
                nc.vector.tensor_mul(hT_sb[hb][:, :n0], slu[hb][:], ph3[:, :n0])

            # remaining chunks: fused per-hid-block ph1/ph3
            t0 = n0
            for n in chunks[1:]:
                for hb in range(KH):
                    ph1 = psA.tile([P, TBS], f32, tag="ph1", bufs=4)
                    ph3 = psA.tile([P, TBS], f32, tag="ph3", bufs=2)
                    mm_sweep(ph1, w1_sb, hb, t0, n)
                    mm_sweep(ph3, w3_sb, hb, t0, n)
                    # silu(h1)*h3 = sigmoid(h1)*h1*h3 (Silu isn't in CoreSim)
                    sil = sil_pool.tile([P, TBS], f32, tag="sil")
                    sg2 = sil_pool.tile([P, TBS], f32, tag="sg2")
                    nc.scalar.activation(sil[:, :n], ph1[:, :n], AF.Sigmoid, bias=zb[:])
                    nc.vector.tensor_mul(sg2[:, :n], sil[:, :n], ph1[:, :n])
                    nc.vector.tensor_mul(
                        hT_sb[hb][:, t0:t0 + n], sg2[:, :n], ph3[:, :n]
                    )
                t0 += n

            # phase 2: yT[d, tok] = W2.T @ h, tokens as the moving dim so
            # cost scales with real tokens; combine weight applied elementwise
            # against a host-broadcast [P, C] tile during PSUM eviction (DVE).
            t0 = 0
            for n in chunks:
                for db in range(KD):
                    py = psY.tile([P, TBS], f32, tag="py", bufs=2)
                    for k in range(KH):
                        nc.tensor.matmul(
                            py[:, :n],
                            lhsT=w2_sb[k][:, db * P:(db + 1) * P],
                            rhs=hT_sb[k][:, t0:t0 + n],
                            start=(k == 0),
                            stop=(k == KH - 1),
                        )
                    ysb = y_pool.tile([P, TBS], f32, tag="y")
                    nc.vector.tensor_mul(ysb[:, :n], py[:, :n], wb_sb[:, t0:t0 + n])
                    nc.sync.dma_start(
                        out=out[db * P:(db + 1) * P, t0:t0 + n], in_=ysb[:, :n]
                    )
                t0 += n

    nc.compile()
    return nc


def _get_kernel(C, C_comp):
    key = (C, C_comp)
    nc = _KERNEL_CACHE.get(key)
    if nc is None:
        nc = _build(C, C_comp)
        _KERNEL_CACHE[key] = nc
    return nc


def _route(xt, Wg):
    """Host gate: returns per-expert (token_indices, combine_weights)."""
    scores = xt.astype(np.float32) @ Wg.astype(np.float32)          # [T, E]
    top2 = np.argpartition(-scores, 1, axis=1)[:, :2]               # [T, 2]
    vals = np.take_along_axis(scores, top2, axis=1)                 # [T, 2]
    vals = vals - vals.max(axis=1, keepdims=True)
    ev = np.exp(vals)
    sm = ev / ev.sum(axis=1, keepdims=True)                         # [T, 2]
    T = xt.shape[0]
    combine = np.zeros((T, E), dtype=np.float32)
    combine[np.arange(T)[:, None], top2] = sm
    idx = []
    wts = []
    for e in range(E):
        ie = np.nonzero(combine[:, e])[0]
        idx.append(ie)
        wts.append(combine[ie, e])
    return idx, wts


def kernel(x, Wg, W1, W3, W2):
    global LAST_RESULT
    from concourse import bass_utils

    orig_shape = x.shape
    orig_dtype = x.dtype
    xt = np.ascontiguousarray(np.asarray(x, dtype=np.float32).reshape(-1, DIM))
    T = xt.shape[0]

    idx, wts = _route(xt, np.asarray(Wg, dtype=np.float32))
    max_n = max(len(i) for i in idx)
    C = max(P, -(-max_n // P) * P)
    C_comp = max(1, max_n)
    NT = C // P

    nc = _get_kernel(C, C_comp)

    W1 = np.asarray(W1)
    W3 = np.asarray(W3)
    W2 = np.asarray(W2)
    in_maps = []
    for e in range(E):
        n_e = len(idx[e])
        xT_e = np.zeros((DIM, C), dtype=BF16)
        xT_e[:, :n_e] = np.ascontiguousarray(xt[idx[e]].T).astype(BF16)
        wv_pad = np.zeros(C, dtype=np.float32)
        wv_pad[:n_e] = wts[e]
        wb_e = np.ascontiguousarray(np.broadcast_to(wv_pad, (P, C)))
        in_maps.append(
            {
                "xT": xT_e,
                "w1": W1[e].astype(BF16),
                "w3": W3[e].astype(BF16),
                "w2": W2[e].astype(BF16),
                "wb": wb_e,
            }
        )

    res = bass_utils.run_bass_kernel_spmd(nc, in_maps, core_ids=list(range(NCORES)))
    LAST_RESULT = res

    out = np.zeros((T, DIM), dtype=np.float32)
    for e in range(E):
        n_e = len(idx[e])
        if n_e:
            out[idx[e]] += np.asarray(
                res.results[e]["out"][:, :n_e], dtype=np.float32
            ).T
    return out.reshape(orig_shape).astype(orig_dtype, copy=False)
